# revision 23
# baseline (speedup 1.0000x reference)
"""Trainium2 Bass kernel for nn_AttentiveRelativeRole (vq_codebook).

Math (forward pass):
  q = split((query+q_pos) @ Wq); k = split((key+kv_pos) @ Wk); v = split((value+kv_pos) @ Wv)
  attn = softmax(q @ k^T + mask)          (mask == 0 per the input spec)
  role = attn @ v                          [B, H*Q, DK]
  sim  = l2n(role) @ l2n(codebook)^T
  ids  = argmax(sim); ws = max(sim)
  quantized = LN2(LN1(codebook[ids]) @ Wo)  -> precomputed per-codebook-row
              table, gathered by ids.

Key identities used:
  * softmax denominator cancels inside the cosine similarity, so the kernel
    never materializes or divides by the softmax row sum.
  * ws = max(simU) * rsqrt(sum(roleU^2)) with roleU the unnormalized role.
  * the per-row score max (softmax stability) is injected into the scores
    matmul as one extra contraction row: [kT; ones]^T @ [qT; -m].

Sharding: 8 cores, core -> (batch = core//2, heads = 8*(core%2) .. +8).
"""

import numpy as np
from contextlib import ExitStack

import concourse.bass as bass
import concourse.tile as tile
from concourse import mybir
from concourse.bass_utils import run_bass_kernel_spmd

F32 = mybir.dt.float32
BF16 = mybir.dt.bfloat16
I32 = mybir.dt.int32
U16 = mybir.dt.uint16
I16 = mybir.dt.int16

B, Q, K, D = 4, 1024, 1024, 1024
H, DK = 16, 64
V = 2048
EPS = 1e-6
NCORES = 8
HC = 8            # heads per core
TOK = HC * Q      # tokens per core (8192)
NT = TOK // 128   # token tiles per core (64)


# --------------------------------------------------------------------------
# walrus workaround: CTRL-struct instructions (Drain/Nop) accept only one
# sync-wait command and Tile piles all end-of-context waits onto one Drain.
# Split every instruction carrying >1 wait into single-wait NOPs on the same
# engine (sequencers execute in order -> semantically identical).
# --------------------------------------------------------------------------
def _split_waits(nc, max_waits=1):
    for f in nc.m.functions:
        for blk in f.blocks:
            insts = blk.instructions
            i = 0
            while i < len(insts):
                inst = insts[i]
                si = inst.sync_info
                ow = list(si.on_wait) if si is not None and si.on_wait else []
                if len(ow) > max_waits:
                    hoist, keep = ow[:-max_waits], ow[-max_waits:]
                    for j, w in enumerate(hoist):
                        nop = mybir.InstNoOp(
                            name=f"{inst.name}-wsplit{j}", ins=[], outs=[])
                        nop.engine = inst.engine
                        nop.sync_info = mybir.SyncInfo(on_wait=[w], on_update=[])
                        nc.register_instruction(nop)
                        insts.insert(i, nop)
                        i += 1
                    si.on_wait = keep
                i += 1
    return nc


# --------------------------------------------------------------------------
# device program
# --------------------------------------------------------------------------
def _build_program():
    nc = bass.Bass()

    d_x = {}
    d_w = {}
    for t in ("q", "k", "v"):
        for p in ("hi", "lo"):
            d_x[t, p] = nc.dram_tensor(f"x{t}T_{p}", [D, Q], BF16,
                                       kind="ExternalInput")
            d_w[t, p] = nc.dram_tensor(f"w{t}_{p}", [D, 512], BF16,
                                       kind="ExternalInput")
    d_cbT2 = nc.dram_tensor("cbT2", [128, V], F32, kind="ExternalInput")
    d_tabT2 = nc.dram_tensor("tabT2", [128, V], F32, kind="ExternalInput")

    d_quantT = nc.dram_tensor("quantT", [DK, TOK], F32, kind="ExternalOutput")
    d_ids = nc.dram_tensor("ids", [NT, 128], I32, kind="ExternalOutput")
    d_ws = nc.dram_tensor("ws", [NT, 128], F32, kind="ExternalOutput")

    d_ident = nc.inline_tensor(np.eye(128, dtype=np.float32), "ident")

    with tile.TileContext(nc) as tc:
        with tc.tile_pool(name="const", bufs=1) as cpool, \
             tc.tile_pool(name="main", bufs=1) as mpool, \
             tc.tile_pool(name="psA", bufs=2, space="PSUM") as ppA, \
             tc.tile_pool(name="psB", bufs=1, space="PSUM") as ppB:

            ident = cpool.tile([128, 128], F32, tag="ident")
            nc.sync.dma_start(ident[:], d_ident[:, :])
            cbT2 = cpool.tile([128, V], F32, tag="cbT2")
            nc.sync.dma_start(cbT2[:], d_cbT2[:, :])
            ones64 = cpool.tile([128, 1], F32, tag="ones64")
            nc.vector.memset(ones64[:], 1.0)
            ones512 = cpool.tile([128, 512], F32, tag="ones512")
            nc.vector.memset(ones512[:], 1.0)
            tabT2 = cpool.tile([128, V], F32, tag="tabT2")
            nc.sync.dma_start(tabT2[:], d_tabT2[:, :])

            # persistent activations (split per-tile for fine-grained deps)
            qTm = [mpool.tile([128, Q], F32, tag=f"qT{m}", name=f"qT{m}")
                   for m in range(4)]
            kTm = [mpool.tile([128, K], F32, tag=f"kT{m}", name=f"kT{m}")
                   for m in range(4)]
            vm = [mpool.tile([128, 512], F32, tag=f"v{t}", name=f"v{t}")
                  for t in range(8)]
            qTbm = [mpool.tile([128, Q], BF16, tag=f"qTb{m}", name=f"qTb{m}")
                    for m in range(4)]
            kTbm = [mpool.tile([128, K], BF16, tag=f"kTb{m}", name=f"kTb{m}")
                    for m in range(4)]

            m_allh = [mpool.tile([128, NT // 2], F32, tag=f"m_all{i}",
                                 name=f"m_all{i}") for i in range(2)]
            ids_f32h = [mpool.tile([128, NT // 2], F32, tag=f"ids_f32{i}",
                                   name=f"ids_f32{i}") for i in range(2)]
            idx16 = mpool.tile([128, TOK // 16], U16, tag="idx16")
            ssq8h = [mpool.tile([HC // 2, Q], F32, tag=f"ssq8{i}",
                                name=f"ssq8{i}") for i in range(2)]

            # ---------------- phase 1: QKV projections (fp32) ----------------
            with tc.tile_pool(name="stage0", bufs=1) as s0:
                def load_xw(t):
                    xs, ws_ = [], []
                    for c in range(8):
                        xh = s0.tile([128, Q], BF16, tag=f"xh{c}", name=f"xh{c}")
                        nc.sync.dma_start(xh[:], d_x[t, "hi"][c*128:(c+1)*128, :])
                        xl = s0.tile([128, Q], BF16, tag=f"xl{c}", name=f"xl{c}")
                        nc.sync.dma_start(xl[:], d_x[t, "lo"][c*128:(c+1)*128, :])
                        wh = s0.tile([128, 512], BF16, tag=f"wh{c}", name=f"wh{c}")
                        nc.sync.dma_start(wh[:], d_w[t, "hi"][c*128:(c+1)*128, :])
                        wl = s0.tile([128, 512], BF16, tag=f"wl{c}", name=f"wl{c}")
                        nc.sync.dma_start(wl[:], d_w[t, "lo"][c*128:(c+1)*128, :])
                        xs.append((xh, xl))
                        ws_.append((wh, wl))
                    return xs, ws_

                def proj_T(dsts, bdsts, xs, ws_):
                    for m in range(4):
                        for nch in range(2):
                            ps = ppA.tile([128, 512], F32, tag="pa", bufs=4)
                            n = 0
                            for c in range(8):
                                xh, xl = xs[c]
                                wh, wl = ws_[c]
                                for wt_, xt_ in ((wh, xh), (wh, xl), (wl, xh)):
                                    nc.tensor.matmul(
                                        ps[:],
                                        wt_[:, m * 128:(m + 1) * 128],
                                        xt_[:, nch * 512:(nch + 1) * 512],
                                        start=(n == 0), stop=(n == 23),
                                        skip_group_check=True)
                                    n += 1
                            nc.scalar.copy(
                                dsts[m][:, nch * 512:(nch + 1) * 512],
                                ps[:])
                        nc.vector.tensor_copy(bdsts[m][:], dsts[m][:])

                xs, ws_ = load_xw("q")
                proj_T(qTm, qTbm, xs, ws_)
                xs, ws_ = load_xw("k")
                proj_T(kTm, kTbm, xs, ws_)
                xs, ws_ = load_xw("v")
                for tt in range(8):
                    ps = ppA.tile([128, 512], F32, tag="pa", bufs=4)
                    n = 0
                    for c in range(8):
                        xh, xl = xs[c]
                        wh, wl = ws_[c]
                        for xt_, wt_ in ((xh, wh), (xh, wl), (xl, wh)):
                            nc.tensor.matmul(
                                ps[:],
                                xt_[:, tt * 128:(tt + 1) * 128],
                                wt_[:],
                                start=(n == 0), stop=(n == 23),
                                skip_group_check=True)
                            n += 1
                    nc.scalar.copy(vm[tt][:], ps[:])

            _wstack = ExitStack()
            wpool = _wstack.enter_context(tc.tile_pool(name="work", bufs=2))

            # ---------------- phase 2: score-max prepass (bf16) --------------
            # head h lives at partitions (h%2)*64.. of mtile h//2
            mtrs = []
            for h in range(HC):
                pb = (h % 2) * 64
                mt = h // 2
                m_h2 = wpool.tile([128, 16], F32, tag="m_h2")
                for qt in range(8):
                    for c in range(2):
                        ps = ppA.tile([128, 512], F32, tag="pa", bufs=4)
                        nc.tensor.matmul(
                            ps[:],
                            qTbm[mt][pb:pb + 64, qt * 128:(qt + 1) * 128],
                            kTbm[mt][pb:pb + 64, c * 512:(c + 1) * 512],
                            start=True, stop=True)
                        nc.vector.tensor_reduce(
                            m_h2[:, qt * 2 + c:qt * 2 + c + 1], ps[:],
                            axis=mybir.AxisListType.X, op=mybir.AluOpType.max)
                m_hn = wpool.tile([128, 8], F32, tag="m_hn")
                nc.vector.tensor_reduce(
                    m_hn[:], m_h2[:].rearrange("p (a c) -> p a c", c=2),
                    axis=mybir.AxisListType.X, op=mybir.AluOpType.max,
                    negate=True)
                pt = ppA.tile([128, 512], F32, tag="pb")
                nc.tensor.transpose(pt[0:8, 0:128], m_hn[:, 0:8], ident[:])
                mtr_h = mpool.tile([8, 128], F32, tag=f"mtr{h}", name=f"mtr{h}")
                nc.vector.tensor_copy(mtr_h[:], pt[0:8, 0:128])
                mtrs.append(mtr_h)

            def emit_outputs(half):
                # rinv for this half's heads
                sqrt4 = wpool.tile([HC // 2, Q], F32, tag="sqrt4", bufs=1)
                nc.scalar.activation(sqrt4[:], ssq8h[half][:],
                                     mybir.ActivationFunctionType.Sqrt)
                rinv4 = wpool.tile([HC // 2, Q], F32, tag="rinv4", bufs=1)
                nc.vector.reciprocal(rinv4[:], sqrt4[:])
                rinvTh = wpool.tile([NT // 2, 128], F32, tag="rinvTh")
                nc.sync.dma_start(
                    rinvTh[:], rinv4[:].rearrange("h (a p) -> h a p", p=128))

                pt = ppA.tile([128, 512], F32, tag="pb")
                nc.tensor.transpose(pt[0:32, 0:128], ids_f32h[half][:, 0:32],
                                    ident[:])
                idsT = wpool.tile([NT // 2, 128], I32, tag="idsT")
                nc.vector.tensor_copy(idsT[:], pt[0:32, 0:128])
                nc.sync.dma_start(
                    d_ids[half * 32:(half + 1) * 32, :], idsT[:])
                ids16T = wpool.tile([NT // 2, 128], U16, tag="ids16T")
                nc.vector.tensor_copy(ids16T[:], pt[0:32, 0:128])

                pt2 = ppA.tile([128, 512], F32, tag="pb")
                nc.tensor.transpose(pt2[0:32, 0:128], m_allh[half][:, 0:32],
                                    ident[:])
                wsT = wpool.tile([NT // 2, 128], F32, tag="wsT")
                nc.vector.tensor_mul(wsT[:], pt2[0:32, 0:128], rinvTh[:])
                nc.sync.dma_start(d_ws[half * 32:(half + 1) * 32, :], wsT[:])

                # gather indices for this half: idx16[q, 8a+r] = ids16T[a, 16r+q]
                hs = slice(half * 256, (half + 1) * 256)
                for qq in range(16):
                    nc.sync.dma_start(
                        idx16[qq:qq + 1, hs].rearrange(
                            "o (a r) -> o a r", r=8),
                        ids16T[:, qq::16])
                for rep in range(1, 8):
                    nc.sync.dma_start(
                        idx16[rep * 16:(rep + 1) * 16, hs], idx16[0:16, hs])

                quantRep = wpool.tile([128, TOK // 2], F32, tag="quantRep",
                                      bufs=1, name=f"quantRep{half}")
                for ch in range(4):
                    gch = half * 4 + ch
                    nc.gpsimd.indirect_copy(
                        quantRep[:, ch * 1024:(ch + 1) * 1024], tabT2[:],
                        idx16[:, gch * 64:(gch + 1) * 64],
                        i_know_ap_gather_is_preferred=True)
                nc.sync.dma_start(
                    d_quantT[:, half * 4096:(half + 1) * 4096],
                    quantRep[0:DK, :])

            # ---------------- phases 3-6 per head pair -----------------------
            for g in range(4):
                hA, hB = 2 * g, 2 * g + 1
                ext = {}
                for h in (hA, hB):
                    pb = (h % 2) * 64
                    mt = h // 2
                    qx = wpool.tile([65, Q], F32, tag=f"qext{h % 2}")
                    nc.sync.dma_start(qx[0:64, :], qTm[mt][pb:pb + 64, :])
                    nc.sync.dma_start(
                        qx[64:65, :].rearrange("o (a p) -> o a p", p=128),
                        mtrs[h][:])
                    kx = wpool.tile([65, K], F32, tag=f"kext{h % 2}")
                    nc.sync.dma_start(kx[0:64, :], kTm[mt][pb:pb + 64, :])
                    nc.vector.memset(kx[64:65, :], 1.0)
                    ext[h] = (qx, kx)

                av = [ppB.tile([128, 512], F32, tag=f"av{c}", name=f"av{c}") for c in range(2)]
                for kt in range(8):
                    pts = {}
                    for h in (hA, hB):
                        qx, kx = ext[h]
                        ptile = wpool.tile([128, K], F32, tag=f"pt{h % 2}")
                        for c in range(2):
                            ps = ppA.tile([128, 512], F32, tag="pb")
                            nc.tensor.matmul(
                                ps[:],
                                kx[:, kt * 128:(kt + 1) * 128],
                                qx[:, c * 512:(c + 1) * 512],
                                start=True, stop=True)
                            nc.scalar.activation(
                                ptile[:, c * 512:(c + 1) * 512], ps[:],
                                mybir.ActivationFunctionType.Exp)
                        pts[h] = ptile
                    for c in range(2):
                        nc.tensor.matmul(
                            av[c][0:64, :],
                            vm[kt][:, hA * 64:(hA + 1) * 64],
                            pts[hA][:, c * 512:(c + 1) * 512],
                            start=(kt == 0), stop=(kt == 7),
                            tile_position=(0, 0), skip_group_check=True)
                        nc.tensor.matmul(
                            av[c][64:128, :],
                            vm[kt][:, hB * 64:(hB + 1) * 64],
                            pts[hB][:, c * 512:(c + 1) * 512],
                            start=(kt == 0), stop=(kt == 7),
                            tile_position=(0, 64), skip_group_check=True)

                roleT = wpool.tile([128, Q], F32, tag="roleT")
                for c in range(2):
                    nc.scalar.copy(
                        roleT[:, c * 512:(c + 1) * 512], av[c][:])

                # ssq (for ws only)
                sq = wpool.tile([128, Q], F32, tag="sq", bufs=1)
                nc.vector.tensor_mul(sq[:], roleT[:], roleT[:])
                for h, pb in ((hA, 0), (hB, 64)):
                    for c in range(2):
                        ps = ppA.tile([128, 512], F32, tag="pb")
                        nc.tensor.matmul(
                            ps[0:1, :],
                            ones64[pb:pb + 64, :],
                            sq[pb:pb + 64, c * 512:(c + 1) * 512],
                            start=True, stop=True,
                            tile_position=(pb, 0))
                        sst = wpool.tile([1, 512], F32, tag="sst")
                        nc.vector.tensor_copy(sst[:], ps[0:1, :])
                        nc.sync.dma_start(
                            ssq8h[h // 4][h % 4:h % 4 + 1,
                                          c * 512:(c + 1) * 512], sst[:])

                # ------------- VQ similarity + argmax ------------------------
                for qt in range(8):
                    for h, pb in ((hA, 0), (hB, 64)):
                        ttile = h * 8 + qt
                        sim = wpool.tile([128, V], F32, tag="sim", bufs=2)
                        for c in range(4):
                            ps = ppA.tile([128, 512], F32, tag="pa", bufs=4)
                            nc.tensor.matmul(
                                ps[:],
                                roleT[pb:pb + 64, qt * 128:(qt + 1) * 128],
                                cbT2[pb:pb + 64, c * 512:(c + 1) * 512],
                                start=True, stop=True,
                                tile_position=(pb, 0))
                            nc.scalar.copy(
                                sim[:, c * 512:(c + 1) * 512], ps[:])
                        half, lt = divmod(ttile, NT // 2)
                        nc.vector.tensor_reduce(
                            m_allh[half][:, lt:lt + 1], sim[:],
                            axis=mybir.AxisListType.X, op=mybir.AluOpType.max)
                        m8 = wpool.tile([128, 8], F32, tag="m8")
                        nc.vector.tensor_scalar(
                            m8[:], ones512[:, 0:8],
                            m_allh[half][:, lt:lt + 1], None,
                            op0=mybir.AluOpType.mult)
                        idx8 = wpool.tile([128, 8], U16, tag="idx8")
                        nc.vector.max_index(idx8[:], m8[:], sim[:])
                        nc.vector.tensor_copy(
                            ids_f32h[half][:, lt:lt + 1], idx8[:, 0:1])

                if g % 2 == 1:
                    emit_outputs(g // 2)

            _wstack.close()

    _split_waits(nc)
    return nc


_CACHED = {}


def _get_program():
    if "nc" not in _CACHED:
        _CACHED["nc"] = _build_program()
    return _CACHED["nc"]


# --------------------------------------------------------------------------
# host glue
# --------------------------------------------------------------------------
def _ln_np(x, g, b):
    m = x.mean(-1, keepdims=True)
    v = x.var(-1, keepdims=True)
    return (x - m) / np.sqrt(v + EPS) * g + b


def kernel(query, key, value, mask, q_pos, kv_pos, Wq, Wk, Wv,
           codebook, vq_ln_g, vq_ln_b, Wo, ln_g, ln_b, **_ignored):
    query = np.asarray(query, np.float32)
    key = np.asarray(key, np.float32)
    value = np.asarray(value, np.float32)
    q_pos = np.asarray(q_pos, np.float32)
    kv_pos = np.asarray(kv_pos, np.float32)
    Wq = np.asarray(Wq, np.float32)
    Wk = np.asarray(Wk, np.float32)
    Wv = np.asarray(Wv, np.float32)
    codebook = np.asarray(codebook, np.float32)

    # host prep: codebook-derived tables (tiny: 2048x64)
    cbn = codebook / np.sqrt(
        np.maximum((codebook * codebook).sum(-1, keepdims=True), 1e-12))
    cbT2 = np.ascontiguousarray(
        np.concatenate([cbn.T, cbn.T], axis=0))            # [128, V]
    table = _ln_np(_ln_np(codebook, np.asarray(vq_ln_g, np.float32),
                          np.asarray(vq_ln_b, np.float32))
                   @ np.asarray(Wo, np.float32),
                   np.asarray(ln_g, np.float32), np.asarray(ln_b, np.float32))
    tabT2 = np.ascontiguousarray(
        np.concatenate([table.T, table.T], axis=0))        # [128, V]

    xq = query + q_pos
    xk = key + kv_pos
    xv = value + kv_pos

    def hilo(a):
        hi = a.astype(np.dtype("bfloat16") if hasattr(np, "bfloat16") else None) \
            if False else None
        import ml_dtypes
        hi = a.astype(ml_dtypes.bfloat16)
        lo = (a - hi.astype(np.float32)).astype(ml_dtypes.bfloat16)
        return np.ascontiguousarray(hi), np.ascontiguousarray(lo)

    in_maps = []
    for core in range(NCORES):
        b = core // 2
        h0 = (core % 2) * HC
        sl = slice(h0 * DK, (h0 + HC) * DK)
        im = {"cbT2": cbT2, "tabT2": tabT2}
        for t, xmat, wmat in (("q", xq[b].T, Wq[:, sl]),
                              ("k", xk[b].T, Wk[:, sl]),
                              ("v", xv[b].T, Wv[:, sl])):
            xh, xl = hilo(np.ascontiguousarray(xmat))
            wh, wl = hilo(np.ascontiguousarray(wmat))
            im[f"x{t}T_hi"], im[f"x{t}T_lo"] = xh, xl
            im[f"w{t}_hi"], im[f"w{t}_lo"] = wh, wl
        in_maps.append(im)

    nc = _get_program()
    res = run_bass_kernel_spmd(nc, in_maps, core_ids=list(range(NCORES)))

    quant = np.zeros((B, H, Q, DK), np.float32)
    ids = np.zeros((B, H, Q, 1), np.int32)
    ws = np.zeros((B, H, Q, 1), np.float32)
    for core in range(NCORES):
        b = core // 2
        h0 = (core % 2) * HC
        r = res.results[core]
        quant[b, h0:h0 + HC] = r["quantT"].T.reshape(HC, Q, DK)
        ids[b, h0:h0 + HC, :, 0] = r["ids"].reshape(HC, Q)
        ws[b, h0:h0 + HC, :, 0] = r["ws"].reshape(HC, Q)
    return quant, ids, ws
